# revision 9
# baseline (speedup 1.0000x reference)
"""CLIP attention (ShareKey branch) Trainium2 Bass kernel, 8-core SPMD. v2.

Math: attn = softmax(scores[..., None] + share_bias, axis=-1) where scores is
constant along the softmax axis -> shift invariance kills the q-projection /
share_key entirely. Remaining computation, per batch b and head h:

    P[h]   = softmax(share_bias[h], axis=-1)        (batch independent)
    V[b]   = hidden[b] @ v_w.T + v_b
    O[b,h] = P[h] @ V[b,h]
    out[b] = concat_h(O[b,h]) @ out_w.T + out_b

v_b is folded into V before attention (P rows sum to 1 -> exact).

Sharding: data-parallel over batch (16/8 = 2 per core), weights + bias
replicated. Host does layout only: transposes, bf16 casts, bias^T padded to
640 rows with -30 (exp -> ~0, so the ragged 577 j-dim becomes 5 clean
128-partition K tiles).

Cost-model-driven design (matmul cost = moving-dim size only, indep of M/K):
  - all matmuls bf16 (1.0 cycles/row at any N)
  - attention per head: lhsT = V[j, (b0 d | b1 d)] (full M=128, both
    batches), rhs = P^T[j, i] streamed N=577  -> 16*5*577 = 46k cycles
    (vs 92k for per-batch M=64, and no separate i-tiling loss)
  - sumexp: P^T j-tiles tree-summed on DVE (4 adds/head), then ONE
    ones-matmul per head -> psum rows all equal r[i]: the reciprocal is
    self-broadcast across partitions, no gpsimd broadcast / row hops
    -> 16*577 = 9.2k cycles instead of 46k of ones-matmuls
  - normalize muls write O^T[hd, i] tiles directly (DVE supports operand
    partition-offset crossing; verified empirically)
Engine budget per core: PE ~219k cycles ~= 91 us; DVE/ACT/Pool all < ~55 us.
"""

import numpy as np

B, S, E = 16, 577, 1024
H, D = 16, 64
NCORES = 8
BPC = B // NCORES  # batches per core

SP = 640  # padded j extent (5 * 128)
NJT = 5  # j tiles (K tiles of attention), last holds 65 valid rows
JSZ = [128, 128, 128, 128, 65]
STILES = [(0, 128), (128, 256), (256, 384), (384, 512), (512, 577)]
NKT = E // 128  # 8 contraction tiles for the projections
NEC = E // 512  # 2 free-dim chunks of the projections
ICH = [(0, 512), (512, 577)]  # moving-dim chunks (psum bank = 512 fp32)


def _build_program():
    import concourse.bass as bass
    import concourse.bacc as bacc
    import concourse.mybir as mybir
    import concourse.tile as tile

    dt = mybir.dt
    f32 = dt.float32
    bf16 = dt.bfloat16
    Exp = mybir.ActivationFunctionType.Exp
    PSUM = bass.MemorySpace.PSUM

    nc = bacc.Bacc("TRN2", target_bir_lowering=False, debug=False, num_devices=NCORES)

    hT = nc.declare_dram_parameter("hiddenT", [BPC, E, S], bf16, isOutput=False)
    wvT = nc.declare_dram_parameter("wvT", [E, E], bf16, isOutput=False)
    woT = nc.declare_dram_parameter("woT", [E, E], bf16, isOutput=False)
    vb = nc.declare_dram_parameter("v_b", [E], f32, isOutput=False)
    ob = nc.declare_dram_parameter("out_b", [E], f32, isOutput=False)
    bT = nc.declare_dram_parameter("biasT", [H, SP, S], bf16, isOutput=False)
    out = nc.declare_dram_parameter("out", [BPC, S, E], f32, isOutput=True)

    with tile.TileContext(nc) as tc:
        with (
            tc.tile_pool(name="const", bufs=1) as const_pool,
            tc.tile_pool(name="vsb", bufs=NJT) as v_pool,
            tc.tile_pool(name="ptp", bufs=H) as pt_pool,
            tc.tile_pool(name="psum", bufs=2, space=PSUM) as psum_pool,
        ):
            # ---- constants ------------------------------------------------
            ones_mat = const_pool.tile([128, 128], bf16, tag="ones", name="ones")
            nc.vector.memset(ones_mat[:], 1.0)
            stage_row = const_pool.tile([1, E], f32, tag="srow", name="srow")
            vb_bc = const_pool.tile([128, E], f32, tag="vbb", name="vbb")
            ob_bc = const_pool.tile([128, E], f32, tag="obb", name="obb")
            nc.sync.dma_start(stage_row[:], vb.rearrange("(a e) -> a e", a=1))
            nc.gpsimd.partition_broadcast(vb_bc[:], stage_row[:])
            nc.sync.dma_start(stage_row[:], ob.rearrange("(a e) -> a e", a=1))
            nc.gpsimd.partition_broadcast(ob_bc[:], stage_row[:])

            # V in SBUF: [j-part, h, (b0 d | b1 d)] bf16 per j-tile, so the
            # attention lhsT (stationary) slice is a single free dim of 128
            v_sb = [
                v_pool.tile([128, H, BPC * D], bf16, tag="v", name="v")
                for _ in range(NJT)
            ]

            # P^T tiles per head: [j-part, jt, i] bf16 (exp applied in place)
            pt_t = []
            for h in range(H):
                p = pt_pool.tile([128, NJT, S], bf16, tag="pt", name="pt")
                # bias DMAs on the ACT queue: keeps SP free for V-proj inputs,
                # and exp (also ACT) pipelines naturally behind each transfer.
                nc.scalar.dma_start(
                    p[:, :, :], bT[h].rearrange("(jt p) i -> p jt i", p=128)
                )
                nc.scalar.activation(p[:, :, :], p[:, :, :], Exp)
                pt_t.append(p)

            # ---- phase A: V projection (bf16), V = hs @ v_w.T + v_b -------
            with (
                tc.tile_pool(name="wvp", bufs=1) as wv_pool,
                tc.tile_pool(name="htp", bufs=BPC) as ht_pool,
            ):
                wv_t = wv_pool.tile([128, NKT, E], bf16, tag="wv", name="wv")
                ht_t = [
                    ht_pool.tile([128, NKT, S], bf16, tag="ht", name="ht")
                    for _ in range(BPC)
                ]
                # ec-half wv loads so the first accumulation chain starts
                # after ~2.2 MB instead of the full weight matrix
                nc.sync.dma_start(
                    wv_t[:, :, 0:512],
                    wvT[:, 0:512].rearrange("(kt p) e -> p kt e", p=128),
                )
                for b in range(BPC):
                    nc.sync.dma_start(
                        ht_t[b][:, :, :], hT[b].rearrange("(kt p) s -> p kt s", p=128)
                    )
                nc.sync.dma_start(
                    wv_t[:, :, 512:1024],
                    wvT[:, 512:1024].rearrange("(kt p) e -> p kt e", p=128),
                )

                for ec in range(NEC):
                    for b in range(BPC):
                        for st, (s0, s1) in enumerate(STILES):
                            ssz = s1 - s0
                            ps = psum_pool.tile(
                                [128, 512], f32, tag="vps", name="vps", bufs=2
                            )
                            for kt in range(NKT):
                                nc.tensor.matmul(
                                    ps[0:ssz, :],
                                    ht_t[b][:, kt, s0:s1],
                                    wv_t[:, kt, bass.ts(ec, 512)],
                                    start=(kt == 0),
                                    stop=(kt == NKT - 1),
                                )
                            nc.vector.tensor_add(
                                v_sb[st][0:ssz, ec * 8 : (ec + 1) * 8, b * D : (b + 1) * D],
                                ps[0:ssz, :],
                                vb_bc[0:ssz, bass.ts(ec, 512)],
                            )

            # B/C-phase pools open after the A-phase pools freed their space
            with (
                tc.tile_pool(name="wop", bufs=1) as wo_pool,
                tc.tile_pool(name="pts", bufs=2) as pts_pool,
                tc.tile_pool(name="invp", bufs=2) as inv_pool,
                tc.tile_pool(name="otp", bufs=BPC * NKT) as ot_pool,
                tc.tile_pool(name="osbp", bufs=2) as osb_pool,
            ):
                _phases_bc(
                    nc, bass, tc, f32, bf16, psum_pool, wo_pool, pts_pool,
                    inv_pool, ot_pool, osb_pool, ones_mat, ob_bc, v_sb, pt_t,
                    woT, out,
                )

    nc.finalize()
    return nc


def _phases_bc(
    nc, bass, tc, f32, bf16, psum_pool, wo_pool, pts_pool, inv_pool, ot_pool,
    osb_pool, ones_mat, ob_bc, v_sb, pt_t, woT, out,
):
    if True:
        if True:
            # out-proj weights load (SP queue, after V-proj inputs)
            wo_t = wo_pool.tile([128, NKT, E], bf16, tag="wo", name="wo")
            nc.sync.dma_start(
                wo_t[:, :, 0:512], woT[:, 0:512].rearrange("(kt p) e -> p kt e", p=128)
            )
            nc.sync.dma_start(
                wo_t[:, :, 512:1024],
                woT[:, 512:1024].rearrange("(kt p) e -> p kt e", p=128),
            )

            # O^T tiles for the out-projection: [hd-pair, i] bf16 per (b, kt)
            ot_t = {}
            for b in range(BPC):
                for kt in range(NKT):
                    ot_t[b, kt] = ot_pool.tile([128, S], bf16, tag="ot", name="ot")

            # ---- phase B: per-head sumexp + attention ---------------------
            for h in range(H):
                kt, half = h // 2, h % 2

                # sumexp: tree-sum the 5 j-tiles (pad rows are exp(-30)~=0),
                # then one ones-matmul -> every psum row equals r[i]
                ptsum = pts_pool.tile([128, S], bf16, tag="pts", name="pts")
                nc.vector.tensor_add(ptsum[:], pt_t[h][:, 0, :], pt_t[h][:, 1, :])
                nc.vector.tensor_add(ptsum[:], ptsum[:], pt_t[h][:, 2, :])
                nc.vector.tensor_add(ptsum[:], ptsum[:], pt_t[h][:, 3, :])
                nc.vector.tensor_add(ptsum[:], ptsum[:], pt_t[h][:, 4, :])

                inv_bc = inv_pool.tile([128, S], f32, tag="inv", name="inv")
                for ci, (i0, i1) in enumerate(ICH):
                    isz = i1 - i0
                    rps = psum_pool.tile(
                        [128, isz], f32, tag=f"rps{ci}", name="rps", bufs=1
                    )
                    nc.tensor.matmul(
                        rps[:, :], ones_mat[:], ptsum[:, i0:i1], start=True, stop=True
                    )
                    nc.vector.reciprocal(inv_bc[:, i0:i1], rps[:, :])

                # attention: psum[(b d), i] accumulated over j tiles
                aps = {}
                for ci, (i0, i1) in enumerate(ICH):
                    isz = i1 - i0
                    aps[ci] = psum_pool.tile(
                        [128, isz], f32, tag=f"aps{ci}", name="aps", bufs=2
                    )
                    for jt in range(NJT):
                        jsz = JSZ[jt]
                        nc.tensor.matmul(
                            aps[ci][:, :],
                            v_sb[jt][0:jsz, h, :],
                            pt_t[h][0:jsz, jt, i0:i1],
                            start=(jt == 0),
                            stop=(jt == NJT - 1),
                        )

                # normalize + scatter: ot[b, kt][half*64 + d, i]
                for b in range(BPC):
                    for ci, (i0, i1) in enumerate(ICH):
                        nc.vector.tensor_mul(
                            ot_t[b, kt][half * 64 : half * 64 + 64, i0:i1],
                            aps[ci][b * 64 : b * 64 + 64, :],
                            inv_bc[b * 64 : b * 64 + 64, i0:i1],
                        )

            # ---- phase C: output projection -------------------------------
            for b in range(BPC):
                for st, (s0, s1) in enumerate(STILES):
                    ssz = s1 - s0
                    osb = osb_pool.tile([128, E], f32, tag="osb", name="osb")
                    for mc in range(NEC):
                        ps = psum_pool.tile(
                            [128, 512], f32, tag="vps", name="ops", bufs=2
                        )
                        for kt in range(NKT):
                            nc.tensor.matmul(
                                ps[0:ssz, :],
                                ot_t[b, kt][:, s0:s1],
                                wo_t[:, kt, bass.ts(mc, 512)],
                                start=(kt == 0),
                                stop=(kt == NKT - 1),
                            )
                        nc.vector.tensor_add(
                            osb[0:ssz, bass.ts(mc, 512)],
                            ps[0:ssz, :],
                            ob_bc[0:ssz, bass.ts(mc, 512)],
                        )
                    nc.sync.dma_start(out[b, s0:s1, :], osb[0:ssz, :])


_NC_CACHE = None


def _get_program():
    global _NC_CACHE
    if _NC_CACHE is None:
        _NC_CACHE = _build_program()
    return _NC_CACHE


def kernel(
    hidden_states,
    q_w,
    q_b,
    v_w,
    v_b,
    out_w,
    out_b,
    share_key,
    share_bias,
    layer,
    _trace=False,
):
    """Full-input / full-output entry point. q_w/q_b/share_key/layer are
    mathematically irrelevant (softmax shift invariance) and unused."""
    import ml_dtypes
    from concourse.bass_utils import run_bass_kernel_spmd

    bf16 = ml_dtypes.bfloat16
    hidden_states = np.asarray(hidden_states, dtype=np.float32)
    v_w = np.asarray(v_w, dtype=np.float32)
    v_b = np.ascontiguousarray(np.asarray(v_b, dtype=np.float32))
    out_w = np.asarray(out_w, dtype=np.float32)
    out_b = np.ascontiguousarray(np.asarray(out_b, dtype=np.float32))
    share_bias = np.asarray(share_bias, dtype=np.float32)

    # host-side layout only: transposes, bf16 casts, j-dim padding
    hiddenT = np.ascontiguousarray(hidden_states.transpose(0, 2, 1)).astype(bf16)
    wvT = np.ascontiguousarray(v_w.T).astype(bf16)
    woT = np.ascontiguousarray(out_w.T).astype(bf16)
    biasT = np.full((H, SP, S), -30.0, dtype=np.float32)
    biasT[:, :S, :] = share_bias.transpose(0, 2, 1)
    biasT = biasT.astype(bf16)

    nc = _get_program()
    in_maps = []
    for c in range(NCORES):
        in_maps.append(
            {
                "hiddenT": hiddenT[c * BPC : (c + 1) * BPC],
                "wvT": wvT,
                "woT": woT,
                "v_b": v_b,
                "out_b": out_b,
                "biasT": biasT,
            }
        )
    res = run_bass_kernel_spmd(nc, in_maps, list(range(NCORES)), trace=_trace)
    out = np.concatenate([res.results[c]["out"] for c in range(NCORES)], axis=0)
    if _trace:
        kernel.last_results = res
    return out
